# revision 8
# baseline (speedup 1.0000x reference)
"""Multi-head self-attention (B=4, L=2048, C=512, NH=8) on 8 Trainium2 cores.

Sharding: core c = 2*b + g owns batch b and head-group g (4 of the 8 heads,
organized as 2 head-PAIRS).  Each core computes QKV for its heads over the
full sequence, full attention for its 4 heads, and a partial output
projection through its rows of w_proj; the two head-group partials per batch
are summed on the host, which also adds b_proj.

x arrives feature-major directly: XT tiles are loaded with transposing
(strided) DMA access patterns straight from DRAM (contiguous 256B chunks, a
plain DMA_DIRECT2D - not the xbar transpose path), so no PE transposes are
needed at all.

QT/KT are [dims, seq] with a head-PAIR packed into one 128-partition tile
(head-even on partitions 0-63, head-odd on 64-127) so the two heads' score
matmuls land in different PE row groups (PE array tiling) and overlap.

Attention runs as 8 streams = (pair, 512-wide q-chunk).  Per kt-PAIR
iteration: 4 score matmuls (2 kt x 2 heads) write two [128, 2, 512] PSUM
tiles (stE/stO); ONE 1024-wide exp per head covers its kt-pair (halving ACT
per-instruction overhead vs 512-wide); AV accumulates into av[65,2,512] with
a ones-column appended to V giving the softmax denominator for free.

Softmax normalization avoids any DRAM round-trip: reciprocal of the rowsum
row [1,2,512] on DVE, replicate across partitions with
gpsimd.partition_broadcast, then one multiply per head on DVE.

PSUM budget (8 banks): stE 2 + stO 2 + av 2 + filler pool 2.
All non-attention matmuls (QKV chunks, V tiles, projection units) run
through the 2-slot filler pool, spread across stream iterations on a
deadline schedule (every filler is emitted in program order BEFORE its first
in-stream consumer - the PE executes in order, so a consumer queued ahead of
its producer would deadlock).
"""

import numpy as np

import concourse.bacc as bacc
import concourse.bass as bass
import concourse.mybir as mybir
import concourse.tile as tile
from concourse import bass_utils

B, L, C, NH, HD = 4, 2048, 512, 8, 64
P = 128
NCORES = 8
GH = NH // 2        # heads per core = 4
GC = GH * HD        # group channels = 256
NCI = C // P        # c_in tiles = 4
NKT = L // P        # k tiles = 16
QC = 512            # q-chunk width
NQC = L // QC       # q chunks = 4
NIT = NKT // 2      # kt-pair iterations per stream = 8

F32 = mybir.dt.float32
BF16 = mybir.dt.bfloat16

EXP = mybir.ActivationFunctionType.Exp


def _build_body(ctx, tc, xb, wg, wp, zt):
    nc = tc.nc

    const = ctx.enter_context(tc.tile_pool(name="const", bufs=1))
    dram = ctx.enter_context(tc.tile_pool(name="dram", bufs=1, space="DRAM"))
    st_ps = ctx.enter_context(tc.tile_pool(name="st_ps", bufs=1, space="PSUM"))
    av_ps = ctx.enter_context(tc.tile_pool(name="av_ps", bufs=1, space="PSUM"))
    fil_ps = ctx.enter_context(tc.tile_pool(name="fil_ps", bufs=2, space="PSUM"))
    epool = ctx.enter_context(tc.tile_pool(name="epool", bufs=6))
    spool = ctx.enter_context(tc.tile_pool(name="spool", bufs=2))
    zpool = ctx.enter_context(tc.tile_pool(name="zpool", bufs=1))

    # Persistent SBUF tensors (feature-major)
    XT = [const.tile([P, 1024], BF16, tag=f"xt{i}", name=f"xt{i}") for i in range(NCI * 2)]
    QT = [const.tile([P, L], BF16, tag=f"qt{p}", name=f"qt{p}") for p in range(2)]
    KT = [const.tile([P, L], BF16, tag=f"kt{p}", name=f"kt{p}") for p in range(2)]
    VA = [const.tile([P, GH * (HD + 1)], BF16, tag=f"va{t}", name=f"va{t}") for t in range(NKT)]
    WGall = const.tile([P, NCI, 3 * GC], BF16, tag="wgall")
    WP4 = const.tile([HD, GH, C], BF16, tag="wp4")
    OT = [[const.tile([HD, QC], BF16, tag=f"ot{h}{c}", name=f"ot{h}{c}") for c in range(NQC)]
          for h in range(GH)]

    # score PSUM tiles: head-even / head-odd, each holding a kt-pair
    stE = st_ps.tile([P, 2, QC], F32, tag="stE", name="stE")
    stO = st_ps.tile([P, 2, QC], F32, tag="stO", name="stO")

    for t in range(NKT):
        va_h = VA[t].rearrange("p (h x) -> p h x", x=HD + 1)
        nc.vector.memset(va_h[:, :, HD : HD + 1], 1.0)

    # PE warm-up: dummy matmuls cover the first x-load DMAs and ramp the PE
    # clock before the real work arrives.
    wtrash = const.tile([P, P], BF16, tag="wtrash")
    nc.vector.memset(wtrash, 0.001)
    ONES1 = const.tile([1, HD], BF16, tag="ones1")
    nc.vector.memset(ONES1, 1.0)
    wps = fil_ps.tile([P, QC], F32, tag="fil", name="warmps")
    for w in range(28):
        nc.tensor.matmul(
            wps[0:HD, 0:P],
            wtrash[:, 0:HD],
            wtrash[:, 0:P],
            start=True,
            stop=True,
            skip_group_check=True,
        )
    wsb = const.tile([1, 8], F32, tag="wsb")
    nc.vector.tensor_copy(out=wsb, in_=wps[0:1, 0:8])

    # ---- loads ----
    # x is loaded directly transposed via the xbar DMA-transpose (bf16, SBUF
    # dest).  All transposes ride the sync queue in s-quad deadline order so
    # no copy-DMA interleaves with them there; the weights ride the gpsimd
    # queue.
    def xt_dma(q, i):
        nc.sync.dma_start(
            out=XT[i * 2 + q // 2][:, (q % 2) * QC : (q % 2 + 1) * QC],
            in_=xb[q * QC : (q + 1) * QC, i * P : (i + 1) * P],
            transpose=True,
        )

    wgr = wg.rearrange("(a p) c -> p a c", p=P)
    nc.gpsimd.dma_start(out=WGall[:, :, 0:GC], in_=wgr[:, :, 0:GC])          # W_q
    nc.gpsimd.dma_start(out=WGall[:, :, GC : 2 * GC], in_=wgr[:, :, GC : 2 * GC])  # W_k
    nc.gpsimd.dma_start(out=WGall[:, :, 2 * GC : 3 * GC], in_=wgr[:, :, 2 * GC : 3 * GC])  # W_v
    nc.gpsimd.dma_start(out=WP4, in_=wp.rearrange("(h p) c -> p h c", p=HD))
    for q in range(4):
        for i in range(NCI):
            xt_dma(q, i)

    # ---- filler units (all through the 2-slot fil_ps pool) ----

    def qkv_fill(dst, p, base, cq, nm):
        """One 512-wide s-chunk of Q^T or K^T for pair p (128 rows = 2 heads).
        NOTE: full-row 128-contraction chains only - alternating PE row
        groups WITHIN one PSUM accumulation group hangs the hardware."""
        ps = fil_ps.tile([P, QC], F32, tag="fil", name=f"qk{nm}")
        for i in range(NCI):
            nc.tensor.matmul(
                ps,
                WGall[:, i, base + p * P : base + (p + 1) * P],
                XT[i * 2 + cq // 2][:, (cq % 2) * QC : (cq % 2 + 1) * QC],
                start=(i == 0),
                stop=(i == NCI - 1),
                skip_group_check=True,
            )
        nc.vector.tensor_copy(out=dst[p][:, cq * QC : (cq + 1) * QC], in_=ps)

    def v_fill(t):
        ps = fil_ps.tile([P, QC], F32, tag="fil", name=f"v{t}")
        for i in range(NCI):
            nc.tensor.matmul(
                ps[:, 0:GC],
                XT[i * 2 + t // 8][:, (t % 8) * P : (t % 8 + 1) * P],
                WGall[:, i, 2 * GC : 3 * GC],
                start=(i == 0),
                stop=(i == NCI - 1),
                skip_group_check=True,
            )
        va_h = VA[t].rearrange("p (h x) -> p h x", x=HD + 1)
        nc.vector.tensor_copy(
            out=va_h[:, :, 0:HD],
            in_=ps[:, 0:GC].rearrange("p (h d) -> p h d", d=HD),
        )

    zparts = {}

    def proj0_fill(c, co):
        """pair-0 half of projection unit (chunk c, out-col block co)."""
        ps = fil_ps.tile([P, QC], F32, tag="fil", name=f"zp0{c}{co}")
        for h in range(2):
            nc.tensor.matmul(
                ps,
                WP4[:, h, co * P : (co + 1) * P],
                OT[h][c],
                start=(h == 0),
                stop=(h == 1),
                skip_group_check=True,
            )
        zs = zpool.tile([P, QC], F32, tag=f"z{c}{co}", name=f"zs{c}{co}")
        nc.vector.tensor_copy(out=zs, in_=ps)
        zparts[(c, co)] = zs

    def projF_fill(c, co):
        """pair-1 half + store of projection unit (chunk c, col block co)."""
        ps = fil_ps.tile([P, QC], F32, tag="fil", name=f"zp1{c}{co}")
        for h in range(2, GH):
            nc.tensor.matmul(
                ps,
                WP4[:, h, co * P : (co + 1) * P],
                OT[h][c],
                start=(h == 2),
                stop=(h == GH - 1),
                skip_group_check=True,
            )
        zf = zpool.tile([P, QC], BF16, tag="zf", name=f"zf{c}{co}", bufs=2)
        nc.vector.tensor_add(out=zf, in0=zparts[(c, co)], in1=ps)
        q = nc.sync if (c + co) % 2 == 0 else nc.gpsimd
        q.dma_start(out=zt[co * P : (co + 1) * P, c * QC : (c + 1) * QC], in_=zf)

    # ---- attention stream: one head-pair x one 512-wide q chunk ----

    def attn_stream(p, c, fillers):
        """fillers: one list of closures per kt-pair iteration; they are
        emitted in the PE-idle window while exp_e runs."""
        qs = slice(c * QC, (c + 1) * QC)
        av = av_ps.tile([HD + 1, 2, QC], F32, tag="av", name=f"av{p}{c}")
        prev = None  # deferred AV_o of previous iteration

        for it in range(NIT):
            kts = (2 * it, 2 * it + 1)
            # scores head-even (PE rows 0-63)
            for j, kt in enumerate(kts):
                nc.tensor.matmul(
                    stE[:, j, :],
                    KT[p][0:HD, kt * P : (kt + 1) * P],
                    QT[p][0:HD, qs],
                    start=True,
                    stop=True,
                    skip_group_check=True,
                )
            e_e = epool.tile([P, 2, QC], BF16, tag="e", name="ee")
            nc.scalar.activation(e_e, stE, EXP, scale=1.0 / np.sqrt(HD))

            # PE-idle window while exp_e runs: fillers, then deferred AV_o
            for f in fillers[it]:
                f()
            if prev is not None:
                prev()
                prev = None

            # scores head-odd (PE rows 64-127)
            for j, kt in enumerate(kts):
                nc.tensor.matmul(
                    stO[:, j, :],
                    KT[p][HD:P, kt * P : (kt + 1) * P],
                    QT[p][HD:P, qs],
                    start=True,
                    stop=True,
                    skip_group_check=True,
                )
            e_o = epool.tile([P, 2, QC], BF16, tag="e", name="eo")
            nc.scalar.activation(e_o, stO, EXP, scale=1.0 / np.sqrt(HD))

            # AV head-even for this iter
            for j, kt in enumerate(kts):
                nc.tensor.matmul(
                    av[:, 0, :],
                    VA[kt][:, 2 * p * (HD + 1) : (2 * p + 1) * (HD + 1)],
                    e_e[:, j, :],
                    start=(it == 0 and j == 0),
                    stop=(it == NIT - 1 and j == 1),
                    skip_group_check=True,
                )

            def av_odd(e_o=e_o, it=it, kts=kts):
                for j, kt in enumerate(kts):
                    nc.tensor.matmul(
                        av[:, 1, :],
                        VA[kt][:, (2 * p + 1) * (HD + 1) : (2 * p + 2) * (HD + 1)],
                        e_o[:, j, :],
                        start=(it == 0 and j == 0),
                        stop=(it == NIT - 1 and j == 1),
                        skip_group_check=True,
                    )

            prev = av_odd

        prev()

        # ---- normalization: no DRAM bounce ----
        # reciprocal of the rowsum row, then replicate across 64 partitions
        # with a ones-stationary matmul (out = ones[1,64].T @ rr[1,512]), then
        # one multiply per head.
        oc = spool.tile([HD + 1, 2, QC], F32, tag="oc", name="oc")
        nc.vector.tensor_copy(out=oc, in_=av)  # frees av for the next stream
        rr = spool.tile([1, 2, QC], BF16, tag="rr", name="rr")
        # bf16 reciprocal of the softmax denominator: 0.4% scale error per
        # query, well inside the output tolerance
        with nc.allow_low_precision(reason="softmax denom reciprocal in bf16"):
            nc.vector.reciprocal(out=rr, in_=oc[HD : HD + 1, :, :])
        for hh in range(2):
            rb = fil_ps.tile([P, QC], F32, tag="fil", name=f"rb{hh}")
            nc.tensor.matmul(
                rb[0:HD, :],
                ONES1[:, 0:HD],
                rr[:, hh, :],
                start=True,
                stop=True,
                skip_group_check=True,
            )
            nc.vector.tensor_mul(
                out=OT[2 * p + hh][c], in0=oc[0:HD, hh, :], in1=rb[0:HD, :]
            )

    # ---- schedule ----
    # pre-stream: q/k chunk 0 for pair 0, first V tiles
    qkv_fill(QT, 0, 0, 0, "q00")
    qkv_fill(KT, 0, GC, 0, "k00")
    for t in range(4):
        v_fill(t)

    def F(*fns):
        return list(fns)

    # HARD deadline rules (PE executes in order - a filler must be emitted
    # before its first consumer):
    #   KT chunk j of the running pair: emitted in an iter < 2j
    #   v_fill(t): emitted in an iter <= t//2 of the FIRST stream of its pair
    #   QT chunk c: emitted before stream (p, c) starts
    sched = {}
    sched[(0, 0)] = [
        F(lambda: qkv_fill(KT, 0, GC, 1, "k01")),
        F(lambda: v_fill(4), lambda: v_fill(5)),
        F(lambda: qkv_fill(KT, 0, GC, 2, "k02"), lambda: v_fill(6)),
        F(lambda: v_fill(7), lambda: v_fill(8)),
        F(lambda: qkv_fill(KT, 0, GC, 3, "k03"), lambda: v_fill(9)),
        F(lambda: v_fill(10), lambda: v_fill(11)),
        F(lambda: v_fill(12), lambda: v_fill(13)),
        F(lambda: v_fill(14), lambda: v_fill(15),
          lambda: qkv_fill(QT, 0, 0, 1, "q01")),
    ]
    sched[(0, 1)] = [
        F(),
        F(),
        F(),
        F(),
        F(),
        F(),
        F(),
        F(lambda: qkv_fill(QT, 0, 0, 2, "q02")),
    ]
    sched[(0, 2)] = [
        F(lambda: proj0_fill(0, 0)),
        F(lambda: proj0_fill(0, 1)),
        F(lambda: proj0_fill(0, 2)),
        F(lambda: proj0_fill(0, 3)),
        F(lambda: proj0_fill(1, 0)),
        F(lambda: proj0_fill(1, 1)),
        F(),
        F(lambda: qkv_fill(QT, 0, 0, 3, "q03")),
    ]
    sched[(0, 3)] = [
        F(lambda: proj0_fill(1, 2)),
        F(lambda: proj0_fill(1, 3)),
        F(lambda: proj0_fill(2, 0)),
        F(lambda: proj0_fill(2, 1)),
        F(lambda: qkv_fill(QT, 1, 0, 0, "q10")),
        F(lambda: qkv_fill(KT, 1, GC, 0, "k10")),
        F(lambda: proj0_fill(2, 2)),
        F(lambda: proj0_fill(2, 3)),
    ]
    sched[(1, 0)] = [
        F(lambda: qkv_fill(KT, 1, GC, 1, "k11")),
        F(lambda: proj0_fill(3, 0)),
        F(lambda: qkv_fill(KT, 1, GC, 2, "k12"), lambda: proj0_fill(3, 1)),
        F(lambda: proj0_fill(3, 2)),
        F(lambda: qkv_fill(KT, 1, GC, 3, "k13"), lambda: proj0_fill(3, 3)),
        F(lambda: qkv_fill(QT, 1, 0, 1, "q11")),
        F(),
        F(),
    ]
    sched[(1, 1)] = [
        F(lambda: projF_fill(0, 0)),
        F(lambda: projF_fill(0, 1)),
        F(lambda: projF_fill(0, 2)),
        F(lambda: projF_fill(0, 3)),
        F(lambda: qkv_fill(QT, 1, 0, 2, "q12")),
        F(),
        F(),
        F(),
    ]
    sched[(1, 2)] = [
        F(lambda: projF_fill(1, 0)),
        F(lambda: projF_fill(1, 1)),
        F(lambda: projF_fill(1, 2)),
        F(lambda: projF_fill(1, 3)),
        F(lambda: qkv_fill(QT, 1, 0, 3, "q13")),
        F(),
        F(),
        F(),
    ]
    sched[(1, 3)] = [
        F(lambda: projF_fill(2, 0)),
        F(lambda: projF_fill(2, 1)),
        F(lambda: projF_fill(2, 2)),
        F(lambda: projF_fill(2, 3)),
        F(),
        F(),
        F(),
        F(),
    ]

    for p in range(2):
        for c in range(NQC):
            attn_stream(p, c, sched[(p, c)])

    # tail: chunk-3 final projection units
    for co in range(NCI):
        projF_fill(3, co)

    # warm-up keep-alive (prevents DCE of the warm-up train; runs at the tail)
    wdr = dram.tile([1, 8], F32, tag="wdr", name="wdr")
    nc.sync.dma_start(out=wdr, in_=wsb)


_CACHE = {}


def _get_nc():
    if "nc" in _CACHE:
        return _CACHE["nc"]
    nc = bacc.Bacc("TRN2", target_bir_lowering=False, debug=False)
    xb = nc.dram_tensor("xb", (L, C), BF16, kind="ExternalInput").ap()
    wg = nc.dram_tensor("wg", (C, 3 * GC), BF16, kind="ExternalInput").ap()
    wp = nc.dram_tensor("wp", (GC, C), BF16, kind="ExternalInput").ap()
    zt = nc.dram_tensor("zt", (C, L), BF16, kind="ExternalOutput").ap()
    from contextlib import ExitStack

    with tile.TileContext(nc) as tc, ExitStack() as ctx:
        _build_body(ctx, tc, xb, wg, wp, zt)
    nc.compile()
    _CACHE["nc"] = nc
    return nc


def make_in_maps(x, w_qkv, w_proj):
    """Slice full inputs into the 8 per-core input maps (pre-cast to bf16)."""
    import ml_dtypes

    bf = ml_dtypes.bfloat16
    x = np.asarray(x, dtype=np.float32).astype(bf)
    w_qkv = np.asarray(w_qkv, dtype=np.float32).astype(bf)
    w_proj = np.asarray(w_proj, dtype=np.float32).astype(bf)
    in_maps = []
    for c in range(NCORES):
        b, g = divmod(c, 2)
        cols = slice(g * GC, (g + 1) * GC)
        wg_c = np.concatenate(
            [w_qkv[:, cols], w_qkv[:, C + g * GC : C + (g + 1) * GC],
             w_qkv[:, 2 * C + g * GC : 2 * C + (g + 1) * GC]],
            axis=1,
        )
        in_maps.append(
            {
                "xb": np.ascontiguousarray(x[b]),
                "wg": np.ascontiguousarray(wg_c),
                "wp": np.ascontiguousarray(w_proj[cols, :]),
            }
        )
    return in_maps


def gather_output(results, b_proj):
    out = np.empty((B, L, C), dtype=np.float32)
    for b in range(B):
        z = (results[2 * b]["zt"].astype(np.float32)
             + results[2 * b + 1]["zt"].astype(np.float32))  # [C, L]
        out[b] = z.T + b_proj[None, :]
    return out


def kernel(x, w_qkv, b_qkv, w_proj, b_proj, _trace=False):
    assert np.abs(np.asarray(b_qkv)).max() == 0.0, "kernel assumes b_qkv == 0"
    nc = _get_nc()
    in_maps = make_in_maps(x, w_qkv, w_proj)
    res = bass_utils.run_bass_kernel_spmd(
        nc, in_maps, core_ids=list(range(NCORES)), trace=_trace
    )
    out = gather_output(res.results, np.asarray(b_proj, dtype=np.float32))
    if _trace:
        return out, res
    return out


# revision 9
# speedup vs baseline: 1.4691x; 1.4691x over previous
"""Multi-head self-attention (B=4, L=2048, C=512, NH=8) on 8 Trainium2 cores.

Sharding: core c = 2*b + g owns batch b and head-group g (4 of the 8 heads).
Each core computes QKV for its heads over the full sequence, full attention
for its 4 heads, and a partial output projection through its rows of w_proj;
the two head-group partials per batch are summed on the host, which also
adds b_proj.

Differences from the old 227us kernel:
  * x is loaded feature-major directly with xbar DMA-transposes (16 tiles on
    the sync queue) - the PE-side transpose fills (and their LDWEIGHTS
    traffic) are gone entirely, and so are the XN staging tiles.
  * Softmax normalization has no DRAM round-trip: reciprocal of the rowsum
    row on DVE (bf16), replicate across 64 partitions with a K=1
    ones-stationary matmul into a PSUM slot, then one DVE multiply.
  * Startup is leaner: weights ride the gpsimd queue in q/k/v pieces while
    the transposes ride sync, and only 2 V tiles are built pre-stream.

Attention core (kept from the old kernel - it is the PSUM-optimal shape):
8 streams = (head, 1024-wide q-chunk), 16 kt iterations each; scores as two
512-col matmuls into a rotating [128,1024] PSUM slot (3-slot pool shared
with all filler work, giving one-iteration score lookahead so ACT stays
fed); one 1024-wide exp per kt on ACT (the pacing engine); AV accumulates
into av[65,1024] with a ones-column appended to V giving the softmax
denominator for free.  QKV/V/projection fill the PE during the ACT-paced
windows on a deadline schedule; every filler is emitted in program order
before its first in-stream consumer (the engines execute in order).

NOTE: alternating PE row groups within one PSUM accumulation group hangs
the hardware - all accumulation chains here stay in a single row group.
"""

import numpy as np

import concourse.bacc as bacc
import concourse.bass as bass
import concourse.mybir as mybir
import concourse.tile as tile
from concourse import bass_utils

B, L, C, NH, HD = 4, 2048, 512, 8, 64
P = 128
NCORES = 8
GH = NH // 2        # heads per core = 4
GC = GH * HD        # group channels = 256
NCI = C // P        # c_in tiles = 4
NKT = L // P        # k tiles = 16

F32 = mybir.dt.float32
BF16 = mybir.dt.bfloat16

EXP = mybir.ActivationFunctionType.Exp


def _build_body(ctx, tc, xb, wg, wp, zt):
    nc = tc.nc

    const = ctx.enter_context(tc.tile_pool(name="const", bufs=1))
    dram = ctx.enter_context(tc.tile_pool(name="dram", bufs=1, space="DRAM"))
    mm_ps = ctx.enter_context(tc.tile_pool(name="mm_ps", bufs=3, space="PSUM"))
    av_ps = ctx.enter_context(tc.tile_pool(name="av_ps", bufs=1, space="PSUM"))
    epool = ctx.enter_context(tc.tile_pool(name="epool", bufs=16))
    spool = ctx.enter_context(tc.tile_pool(name="spool", bufs=2))
    zpool = ctx.enter_context(tc.tile_pool(name="zpool", bufs=1))

    # Persistent SBUF tensors (feature-major)
    XT = [const.tile([P, 1024], BF16, tag=f"xt{i}", name=f"xt{i}") for i in range(NCI * 2)]
    QT = [const.tile([P, L], BF16, tag=f"qt{p}", name=f"qt{p}") for p in range(2)]
    KT = [const.tile([P, L], BF16, tag=f"kt{p}", name=f"kt{p}") for p in range(2)]
    VA = [const.tile([P, GH * (HD + 1)], BF16, tag=f"va{t}", name=f"va{t}") for t in range(NKT)]
    WGall = const.tile([P, NCI, 3 * GC], BF16, tag="wgall")
    WP4 = const.tile([HD, GH, C], BF16, tag="wp4")
    OT = [[const.tile([HD, 1024], BF16, tag=f"ot{h}{c}", name=f"ot{h}{c}") for c in range(2)]
          for h in range(GH)]
    ONES1 = const.tile([1, HD], BF16, tag="ones1")
    nc.vector.memset(ONES1, 1.0)

    for t in range(NKT):
        va_h = VA[t].rearrange("p (h x) -> p h x", x=HD + 1)
        nc.vector.memset(va_h[:, :, HD : HD + 1], 1.0)

    # PE warm-up: dummy matmuls cover the first x-load DMAs and ramp the PE
    # clock before the real work arrives.
    wtrash = const.tile([P, P], BF16, tag="wtrash")
    nc.vector.memset(wtrash, 0.001)
    wps = mm_ps.tile([P, 1024], F32, tag="mm", name="warmps")
    for w in range(40):
        nc.tensor.matmul(
            wps[0:HD, 0:P],
            wtrash[:, 0:HD],
            wtrash[:, 0:P],
            start=True,
            stop=True,
            skip_group_check=True,
        )
    wsb = const.tile([1, 8], F32, tag="wsb")
    nc.vector.tensor_copy(out=wsb, in_=wps[0:1, 0:8])

    # ---- loads ----
    # x feature-major via xbar DMA-transpose, all on the sync queue in s-quad
    # deadline order; weights in q/k/v pieces on the gpsimd queue.
    def xt_dma(q, i):
        nc.sync.dma_start(
            out=XT[i * 2 + q // 2][:, (q % 2) * 512 : (q % 2 + 1) * 512],
            in_=xb[q * 512 : (q + 1) * 512, i * P : (i + 1) * P],
            transpose=True,
        )

    wgr = wg.rearrange("(a p) c -> p a c", p=P)
    nc.gpsimd.dma_start(out=WGall[:, :, 0:GC], in_=wgr[:, :, 0:GC])          # W_q
    nc.gpsimd.dma_start(out=WGall[:, :, GC : 2 * GC], in_=wgr[:, :, GC : 2 * GC])  # W_k
    nc.gpsimd.dma_start(out=WGall[:, :, 2 * GC : 3 * GC], in_=wgr[:, :, 2 * GC : 3 * GC])  # W_v
    nc.gpsimd.dma_start(out=WP4, in_=wp.rearrange("(h p) c -> p h c", p=HD))
    for q in range(4):
        for i in range(NCI):
            xt_dma(q, i)

    # ---- filler units (through the shared 3-slot mm pool) ----

    def qkv_fill(dst, p, base, cq, nm):
        """One 512-wide s-piece of Q^T or K^T for pair p (128 rows = 2 heads)."""
        ps = mm_ps.tile([P, 512], F32, tag="mm", name=f"qk{nm}")
        for i in range(NCI):
            nc.tensor.matmul(
                ps,
                WGall[:, i, base + p * P : base + (p + 1) * P],
                XT[i * 2 + cq // 2][:, (cq % 2) * 512 : (cq % 2 + 1) * 512],
                start=(i == 0),
                stop=(i == NCI - 1),
                skip_group_check=True,
            )
        nc.vector.tensor_copy(out=dst[p][:, cq * 512 : (cq + 1) * 512], in_=ps)

    def v_fill(t):
        ps = mm_ps.tile([P, 512], F32, tag="mm", name=f"v{t}")
        for i in range(NCI):
            nc.tensor.matmul(
                ps[:, 0:GC],
                XT[i * 2 + t // 8][:, (t % 8) * P : (t % 8 + 1) * P],
                WGall[:, i, 2 * GC : 3 * GC],
                start=(i == 0),
                stop=(i == NCI - 1),
                skip_group_check=True,
            )
        va_h = VA[t].rearrange("p (h x) -> p h x", x=HD + 1)
        nc.vector.tensor_copy(
            out=va_h[:, :, 0:HD],
            in_=ps[:, 0:GC].rearrange("p (h d) -> p h d", d=HD),
        )

    zparts = {}

    def proj0_fill(c, co):
        """heads 0-1 half of projection unit (chunk c, out-col block co)."""
        ps = mm_ps.tile([P, 1024], F32, tag="mm", name=f"zp0{c}{co}")
        for h in range(2):
            for half in range(2):
                cols = slice(half * 512, (half + 1) * 512)
                nc.tensor.matmul(
                    ps[:, cols],
                    WP4[:, h, co * P : (co + 1) * P],
                    OT[h][c][:, cols],
                    start=(h == 0),
                    stop=(h == 1),
                    skip_group_check=True,
                )
        zs = zpool.tile([P, 1024], F32, tag=f"z{c}{co}", name=f"zs{c}{co}")
        nc.vector.tensor_copy(out=zs, in_=ps)
        zparts[(c, co)] = zs

    def projF_fill(c, co):
        """heads 2-3 half + store of projection unit (chunk c, col block co)."""
        ps = mm_ps.tile([P, 1024], F32, tag="mm", name=f"zp1{c}{co}")
        for h in range(2, GH):
            for half in range(2):
                cols = slice(half * 512, (half + 1) * 512)
                nc.tensor.matmul(
                    ps[:, cols],
                    WP4[:, h, co * P : (co + 1) * P],
                    OT[h][c][:, cols],
                    start=(h == 2),
                    stop=(h == GH - 1),
                    skip_group_check=True,
                )
        zf = zpool.tile([P, 1024], BF16, tag="zf", name=f"zf{c}{co}", bufs=2)
        nc.vector.tensor_add(out=zf, in0=zparts[(c, co)], in1=ps)
        for half in range(2):
            q = nc.sync if half == 0 else nc.gpsimd
            q.dma_start(
                out=zt[co * P : (co + 1) * P, c * 1024 + half * 512 : c * 1024 + (half + 1) * 512],
                in_=zf[:, half * 512 : (half + 1) * 512],
            )

    # ---- attention stream: one head x one 1024-wide q chunk ----

    def attn_stream(h, ch, fillers):
        p, hh = h // 2, h % 2
        po = hh * HD
        qs = slice(ch * 1024, (ch + 1) * 1024)
        av = av_ps.tile([HD + 1, 1024], F32, tag="av", name=f"av{h}{ch}")
        for kt in range(NKT):
            for f in fillers[kt]:
                f()
            st = mm_ps.tile([P, 1024], F32, tag="mm", name="st")
            for half in range(2):
                hs = slice(half * 512, (half + 1) * 512)
                nc.tensor.matmul(
                    st[:, hs],
                    KT[p][po : po + HD, kt * P : (kt + 1) * P],
                    QT[p][po : po + HD, ch * 1024 + half * 512 : ch * 1024 + (half + 1) * 512],
                    start=True,
                    stop=True,
                    skip_group_check=True,
                )
            e = epool.tile([P, 1024], BF16, tag="e", name="e")
            nc.scalar.activation(e, st, EXP, scale=1.0 / np.sqrt(HD))
            for half in range(2):
                hs = slice(half * 512, (half + 1) * 512)
                nc.tensor.matmul(
                    av[:, hs],
                    VA[kt][:, h * (HD + 1) : (h + 1) * (HD + 1)],
                    e[:, hs],
                    start=(kt == 0),
                    stop=(kt == NKT - 1),
                    skip_group_check=True,
                )

        # ---- normalization: no DRAM bounce ----
        oc = spool.tile([HD + 1, 1024], F32, tag="oc", name="oc")
        nc.vector.tensor_copy(out=oc, in_=av)  # frees av for the next stream
        rr = spool.tile([1, 1024], BF16, tag="rr", name="rr")
        with nc.allow_low_precision(reason="softmax denom reciprocal in bf16"):
            nc.vector.reciprocal(out=rr, in_=oc[HD : HD + 1, :])
        rb = mm_ps.tile([P, 1024], F32, tag="mm", name="rb")
        for half in range(2):
            hs = slice(half * 512, (half + 1) * 512)
            nc.tensor.matmul(
                rb[0:HD, hs],
                ONES1,
                rr[:, hs],
                start=True,
                stop=True,
                skip_group_check=True,
            )
        nc.vector.tensor_mul(out=OT[h][ch], in0=oc[0:HD, :], in1=rb[0:HD, :])

    # ---- schedule ----
    # pre-stream: q/k chunk-0 pieces for pair 0, first V tiles
    qkv_fill(QT, 0, 0, 0, "q0p0")
    qkv_fill(QT, 0, 0, 1, "q0p1")
    qkv_fill(KT, 0, GC, 0, "k0p0")
    v_fill(0)
    v_fill(1)

    def F(*fns):
        return list(fns)

    E8 = [F() for _ in range(8)]

    # stream order: h0c0, h0c1, h1c0, h1c1, h2c0, h2c1, h3c0, h3c1
    # HARD deadlines (engines are in-order; a filler must be emitted before
    # its first consumer):  KT piece j of the running pair before kt 4j;
    # v_fill(t) at/before kt t of the FIRST stream; QT pieces of chunk c
    # before stream (*, c) starts; pair-1 pieces before stream h2c0.
    sched = {}
    sched[(0, 0)] = [
        F(lambda: v_fill(2)),
        F(lambda: v_fill(3)),
        F(lambda: qkv_fill(KT, 0, GC, 1, "k0p1"), lambda: v_fill(4)),
        F(lambda: v_fill(5)),
        F(lambda: v_fill(6)),
        F(lambda: v_fill(7)),
        F(lambda: qkv_fill(KT, 0, GC, 2, "k0p2"), lambda: v_fill(8)),
        F(lambda: v_fill(9)),
        F(lambda: v_fill(10)),
        F(lambda: v_fill(11)),
        F(lambda: qkv_fill(KT, 0, GC, 3, "k0p3"), lambda: v_fill(12)),
        F(lambda: v_fill(13)),
        F(lambda: v_fill(14)),
        F(lambda: v_fill(15)),
        F(lambda: qkv_fill(QT, 0, 0, 2, "q0p2")),
        F(lambda: qkv_fill(QT, 0, 0, 3, "q0p3")),
    ]
    sched[(0, 1)] = [
        F(),
        F(),
        F(lambda: qkv_fill(QT, 1, 0, 0, "q1p0")),
        F(),
        F(lambda: qkv_fill(QT, 1, 0, 1, "q1p1")),
        F(),
        F(lambda: qkv_fill(KT, 1, GC, 0, "k1p0")),
        F(),
        F(lambda: qkv_fill(KT, 1, GC, 1, "k1p1")),
        F(),
        F(lambda: qkv_fill(KT, 1, GC, 2, "k1p2")),
        F(),
        F(lambda: qkv_fill(KT, 1, GC, 3, "k1p3")),
        F(),
        F(lambda: qkv_fill(QT, 1, 0, 2, "q1p2")),
        F(lambda: qkv_fill(QT, 1, 0, 3, "q1p3")),
    ]
    sched[(1, 0)] = [F() for _ in range(16)]
    sched[(1, 1)] = [
        F(), F(), F(),
        F(lambda: proj0_fill(0, 0)),
        F(), F(),
        F(lambda: proj0_fill(0, 1)),
        F(), F(),
        F(lambda: proj0_fill(0, 2)),
        F(), F(),
        F(lambda: proj0_fill(0, 3)),
        F(), F(), F(),
    ]
    sched[(2, 0)] = [
        F(), F(), F(),
        F(lambda: proj0_fill(1, 0)),
        F(), F(),
        F(lambda: proj0_fill(1, 1)),
        F(), F(),
        F(lambda: proj0_fill(1, 2)),
        F(), F(),
        F(lambda: proj0_fill(1, 3)),
        F(), F(), F(),
    ]
    sched[(2, 1)] = [F() for _ in range(16)]
    sched[(3, 0)] = [F() for _ in range(16)]
    sched[(3, 1)] = [
        F(), F(),
        F(lambda: projF_fill(0, 0)),
        F(), F(),
        F(lambda: projF_fill(0, 1)),
        F(), F(),
        F(lambda: projF_fill(0, 2)),
        F(), F(),
        F(lambda: projF_fill(0, 3)),
        F(), F(), F(), F(),
    ]

    for h in range(GH):
        for ch in range(2):
            attn_stream(h, ch, sched[(h, ch)])

    # ---- tail: chunk-1 final units with the h2-first trick ----
    # OT[2][1] has been ready since stream h2c1; only OT[3][1] waits on the
    # last stream's norm.  Issue the h2 matmuls of three units first (they
    # fill the PE during the norm), then stack h3 on each, then the last
    # full unit.
    tail_zp = {}
    for co in range(3):
        ps = mm_ps.tile([P, 1024], F32, tag="mm", name=f"zpt{co}")
        for half in range(2):
            cols = slice(half * 512, (half + 1) * 512)
            nc.tensor.matmul(
                ps[:, cols], WP4[:, 2, co * P : (co + 1) * P], OT[2][1][:, cols],
                start=True, stop=False, skip_group_check=True,
            )
        tail_zp[co] = ps
    for co in range(3):
        ps = tail_zp[co]
        for half in range(2):
            cols = slice(half * 512, (half + 1) * 512)
            nc.tensor.matmul(
                ps[:, cols], WP4[:, 3, co * P : (co + 1) * P], OT[3][1][:, cols],
                start=False, stop=True, skip_group_check=True,
            )
        zf = zpool.tile([P, 1024], BF16, tag="zf", name=f"zft{co}", bufs=2)
        nc.vector.tensor_add(out=zf, in0=zparts[(1, co)], in1=ps)
        for half in range(2):
            q = nc.sync if half == 0 else nc.gpsimd
            q.dma_start(
                out=zt[co * P : (co + 1) * P, 1024 + half * 512 : 1024 + (half + 1) * 512],
                in_=zf[:, half * 512 : (half + 1) * 512],
            )
    projF_fill(1, 3)

    # warm-up keep-alive (prevents DCE of the warm-up train; runs at the tail)
    wdr = dram.tile([1, 8], F32, tag="wdr", name="wdr")
    nc.sync.dma_start(out=wdr, in_=wsb)


_CACHE = {}


def _get_nc():
    if "nc" in _CACHE:
        return _CACHE["nc"]
    nc = bacc.Bacc("TRN2", target_bir_lowering=False, debug=False)
    xb = nc.dram_tensor("xb", (L, C), BF16, kind="ExternalInput").ap()
    wg = nc.dram_tensor("wg", (C, 3 * GC), BF16, kind="ExternalInput").ap()
    wp = nc.dram_tensor("wp", (GC, C), BF16, kind="ExternalInput").ap()
    zt = nc.dram_tensor("zt", (C, L), BF16, kind="ExternalOutput").ap()
    from contextlib import ExitStack

    with tile.TileContext(nc) as tc, ExitStack() as ctx:
        _build_body(ctx, tc, xb, wg, wp, zt)
    nc.compile()
    _CACHE["nc"] = nc
    return nc


def make_in_maps(x, w_qkv, w_proj):
    """Slice full inputs into the 8 per-core input maps (pre-cast to bf16)."""
    import ml_dtypes

    bf = ml_dtypes.bfloat16
    x = np.asarray(x, dtype=np.float32).astype(bf)
    w_qkv = np.asarray(w_qkv, dtype=np.float32).astype(bf)
    w_proj = np.asarray(w_proj, dtype=np.float32).astype(bf)
    in_maps = []
    for c in range(NCORES):
        b, g = divmod(c, 2)
        cols = slice(g * GC, (g + 1) * GC)
        wg_c = np.concatenate(
            [w_qkv[:, cols], w_qkv[:, C + g * GC : C + (g + 1) * GC],
             w_qkv[:, 2 * C + g * GC : 2 * C + (g + 1) * GC]],
            axis=1,
        )
        in_maps.append(
            {
                "xb": np.ascontiguousarray(x[b]),
                "wg": np.ascontiguousarray(wg_c),
                "wp": np.ascontiguousarray(w_proj[cols, :]),
            }
        )
    return in_maps


def gather_output(results, b_proj):
    out = np.empty((B, L, C), dtype=np.float32)
    for b in range(B):
        z = (results[2 * b]["zt"].astype(np.float32)
             + results[2 * b + 1]["zt"].astype(np.float32))  # [C, L]
        out[b] = z.T + b_proj[None, :]
    return out


def kernel(x, w_qkv, b_qkv, w_proj, b_proj, _trace=False):
    assert np.abs(np.asarray(b_qkv)).max() == 0.0, "kernel assumes b_qkv == 0"
    nc = _get_nc()
    in_maps = make_in_maps(x, w_qkv, w_proj)
    res = bass_utils.run_bass_kernel_spmd(
        nc, in_maps, core_ids=list(range(NCORES)), trace=_trace
    )
    out = gather_output(res.results, np.asarray(b_proj, dtype=np.float32))
    if _trace:
        return out, res
    return out
